# revision 9
# baseline (speedup 1.0000x reference)
"""Trainium2 Bass kernel for nn_Model_1245540515968 (gnn_message_passing), v2.

Self-contained: kernel(**inputs) -> np.ndarray [128] per-structure energies.

Strategy (8 cores, graph/data parallel, same algebra as v1 but restructured):
  - Shard by structure: core c owns structures [16c,16c+16) and their atoms.
  - Atoms packed 6-per-tile (128 edge slots); ASPAN=6, TPB=20 tiles/block
    -> 120 atom slots per block, NB blocks.
  - fp16 matmul operands everywhere (PE 1 cyc/row vs fp32's 4).
  - Host-expanded mask mm_exp [P,NT,6,16] fp16 so sh_exp = STT(sh, mm) hits
    the DVE 4x_2p mode (0.26 ns/elem).
  - All elementwise via scalar_tensor_tensor / tensor_scalar (TensorScalarPtr
    -> 2x_2p/4x_2p modes) instead of tensor_tensor (2x_1p only).
  - Newton fast-inverse-sqrt on DVE so Act only needs {sin, square, copy}
    (single act table: trig_and_small; no 1283ns table switches).
  - Phase-1 U matmuls write 4 partition-groups of PSUM (32 rows each), so
    the PSUM->SBUF U copy is [128,480] (484 free elems) not [32,1920].
  - Phase 2: Am = w3_l^T U per (l, g-pair); Act Square PSUM->SBUF fp16;
    DVE pairwise-add tree over m (4x mode); B4/H fp16; per-block energy via
    two small matmuls; fp32 accumulation of per-structure energies.
"""
import os
import sys
from contextlib import ExitStack

import numpy as np

for _p in ("/opt/trn_rl_repo",):
    if _p not in sys.path and os.path.isdir(_p):
        sys.path.insert(0, _p)

import concourse.bass as bass
import concourse.tile as tile
from concourse import bacc, mybir
from concourse.bass_utils import run_bass_kernel_spmd

F32 = mybir.dt.float32
F16 = mybir.dt.float16
I32 = mybir.dt.int32
AX = mybir.AxisListType
OP = mybir.AluOpType
ACTF = mybir.ActivationFunctionType

N_ATOMS = 10000
N_EDGES = 200000
N_SPECIES = 4
N_RAD = 8
N_MAX = [8, 6, 4, 2]
K_MIX = 128
N_STRUCT = 128
CUTOFF = 5.0
N_CORES = 8
S_PER_CORE = N_STRUCT // N_CORES
P = 128
ASPAN = 6           # atoms (lanes) per tile
TPB = 20            # tiles per block
TA = ASPAN * TPB    # atom slots per block = 120
NGRP = 4            # psum partition groups (tl % 4)
CPG = TPB // NGRP   # tiles per group = 5

# sh_full column order: [sh3 (7), l0-const (1), sh1 (3), sh2 (5)]
M_OFF = {3: 0, 0: 7, 1: 8, 2: 11}
M_LEN = {0: 1, 1: 3, 2: 5, 3: 7}

C1 = 0.4886025119029199
C2A = 1.0925484305920792
C2B = 0.31539156525252005
C2C = 0.5462742152960396
C3A = 0.5900435899266435
C3B = 2.890611442640554
C3B2 = 1.445305721320277
C3C = 0.4570457994644658
C3D = 0.3731763325901154
L0C = 0.28209479177387814
MAGIC = 0x5F3759DF


# ----------------------------------------------------------------------------
# Host preprocessing
# ----------------------------------------------------------------------------

def _pack_tiles(atom_ids, degs, nbins):
    """Worst-fit decreasing into a fixed number of bins, each <=ASPAN atoms
    and <=P edge slots. Returns list of tiles (atom-id lists) or None."""
    import heapq
    order = np.argsort(-degs, kind='stable')
    heap = [(-P, j) for j in range(nbins)]
    heapq.heapify(heap)
    used = [0] * nbins
    tiles = [[] for _ in range(nbins)]
    for i in order:
        a, d = int(atom_ids[i]), int(degs[i])
        tmp = []
        placed = False
        while heap:
            negfree, j = heapq.heappop(heap)
            if len(tiles[j]) < ASPAN and used[j] + d <= P:
                used[j] += d
                tiles[j].append(a)
                if len(tiles[j]) < ASPAN:
                    heapq.heappush(heap, (-(P - used[j]), j))
                placed = True
                break
            tmp.append((negfree, j))
            if -negfree < d:
                break
        for it in tmp:
            heapq.heappush(heap, it)
        if not placed:
            return None
    return tiles


def _preprocess(inputs):
    species = np.asarray(inputs['species'])
    senders = np.asarray(inputs['senders'])
    receivers = np.asarray(inputs['receivers'])
    batch_seg = np.asarray(inputs['batch_seg'])
    positions = np.asarray(inputs['positions'], dtype=np.float32)

    struct_starts = np.searchsorted(batch_seg, np.arange(N_STRUCT + 1))
    core_hi = struct_starts[(np.arange(N_CORES) + 1) * S_PER_CORE]
    core_lo = np.concatenate([[0], core_hi[:-1]])

    # receiver-sorted edge index
    r_order = np.argsort(receivers, kind='stable')
    r_sorted = receivers[r_order]
    deg = np.bincount(receivers, minlength=N_ATOMS)
    estart = np.concatenate([[0], np.cumsum(deg)])

    amax = max(int(core_hi[c] - core_lo[c]) for c in range(N_CORES))
    NT = -(-(-(-amax // ASPAN)) // TPB) * TPB
    while True:
        cores = []
        for c in range(N_CORES):
            a_lo, a_hi = int(core_lo[c]), int(core_hi[c])
            aids = np.arange(a_lo, a_hi)
            tiles = _pack_tiles(aids, deg[a_lo:a_hi], NT)
            if tiles is None:
                cores = None
                break
            cores.append(dict(a_lo=a_lo, a_hi=a_hi, tiles=tiles))
        if cores is not None:
            break
        NT += TPB
    NB = NT // TPB

    # weight transforms
    emb = np.asarray(inputs['emb'], np.float32)
    emb2 = np.asarray(inputs['emb2'], np.float32)
    w_out = np.asarray(inputs['w_out'], np.float32)
    scal = float(np.asarray(inputs['scaling'])[0])
    W3 = np.zeros((32, 4 * K_MIX), np.float32)
    for l in range(4):
        w_rad = np.asarray(inputs[f'w_rad{l}'], np.float32) * 0.5  # fcut 0.5
        w_mix = np.asarray(inputs[f'w_mix{l}'], np.float32)
        n_l = N_MAX[l]
        W2 = np.einsum('sc,ri->sric', emb, w_rad).reshape(32, n_l * 16)
        w3 = (W2 @ w_mix) * (2 * l + 1) ** -0.25
        if l == 0:
            w3 = w3 * L0C
        W3[:, l * K_MIX:(l + 1) * K_MIX] = w3
    E2s = (emb2 * w_out[None, :] * scal).astype(np.float32)     # [4, 128]
    cw = np.asarray(inputs['comp_weights'], np.float32)
    cw_struct = np.zeros(N_STRUCT, np.float32)
    np.add.at(cw_struct, batch_seg, cw[species])

    oh_tab = (species[:, None] == np.arange(N_SPECIES)[None, :]).astype(np.float16)

    w3q = np.zeros((P, 16 * K_MIX), np.float32)
    for l in range(4):
        for g in range(NGRP):
            w3q[32 * g:32 * (g + 1), (l * NGRP + g) * K_MIX:
                (l * NGRP + g + 1) * K_MIX] = W3[:, l * K_MIX:(l + 1) * K_MIX]
    shared = dict(w3=np.ascontiguousarray(w3q).astype(np.float16))
    in_maps = []
    for ci, c in enumerate(cores):
        g6 = np.zeros((NT, P, 6), np.float32)
        goh = np.zeros((NT, P, 4), np.float16)
        mm = np.zeros((NT, P, ASPAN), np.float16)
        slot_atom = -np.ones((NB, TA), np.int64)   # [block, ta] -> atom
        for t, tile_atoms in enumerate(c['tiles']):
            b, tl = t // TPB, t % TPB
            g, cc_ = tl % NGRP, tl // NGRP
            s = 0
            for a_local, a in enumerate(tile_atoms):
                d = int(deg[a])
                eds = r_order[estart[a]:estart[a] + d]
                g6[t, s:s + d, 0:3] = positions[senders[eds]]
                g6[t, s:s + d, 3:6] = positions[receivers[eds]]
                goh[t, s:s + d, :] = oh_tab[senders[eds]]
                mm[t, s:s + d, a_local] = 1.0
                ta = g * (CPG * ASPAN) + cc_ * ASPAN + a_local
                slot_atom[b, ta] = a
                s += d
        mm_exp = np.broadcast_to(mm[:, :, :, None], (NT, P, ASPAN, 16))
        m = dict(shared)
        m['g6'] = np.ascontiguousarray(g6.transpose(1, 0, 2).reshape(P, NT * 6))
        m['goh'] = np.ascontiguousarray(goh.transpose(1, 0, 2).reshape(P, NT * 4))
        m['mmx'] = np.ascontiguousarray(
            mm_exp.transpose(1, 0, 2, 3).reshape(P, NT * ASPAN * 16))
        sa = slot_atom.reshape(-1)
        valid = sa >= 0
        sp_slot = np.where(valid, species[np.clip(sa, 0, None)], 0)
        e2full = np.where(valid[None, :], E2s.T[:, sp_slot], 0.0)
        m['e2full'] = np.ascontiguousarray(e2full).astype(np.float16)  # [128, NB*TA]
        S = np.zeros((NB, TA, S_PER_CORE), np.float16)
        bidx = np.nonzero(valid)[0]
        S[bidx // TA, bidx % TA,
          batch_seg[sa[valid]] - ci * S_PER_CORE] = 1.0
        sfull = np.zeros((P, NB * S_PER_CORE), np.float16)
        sfull[:TA, :] = S.transpose(1, 0, 2).reshape(TA, NB * S_PER_CORE)
        m['sstr'] = np.ascontiguousarray(sfull)
        m['einit'] = cw_struct[ci * S_PER_CORE:(ci + 1) * S_PER_CORE].reshape(
            S_PER_CORE, 1).copy()
        m['onesc'] = np.ones((P, 1), np.float16)
        m['zed'] = np.zeros((P, NT * 128), np.float16)
        in_maps.append(m)
    return in_maps, NT, NB


# ----------------------------------------------------------------------------
# Bass program
# ----------------------------------------------------------------------------

def _chunks(NB, n):
    n = min(n, NB)
    base, rem = divmod(NB, n)
    out, b0 = [], 0
    for i in range(n):
        nb = base + (1 if i < rem else 0)
        out.append((b0, b0 + nb))
        b0 += nb
    return out


def _grow_chunks(NB, first=1, mult=2):
    """Geometrically growing chunk sizes: [1, 2, 4, 4, ...] summing to NB."""
    out, b0, sz = [], 0, first
    while b0 < NB:
        nb = min(sz, NB - b0)
        out.append((b0, b0 + nb))
        b0 += nb
        sz = min(sz * mult, 4)
    return out


CFG = dict(nchunks=2, ndma=4, sq_dve=(), geo_pool=('sq3', 'u2', 'prods'),
           pf_eng='pool', b4h_eng='pool', shexp_pool_mod=5)


def build_program(NT, NB, repeat=1):
    cfg = CFG
    nc = bacc.Bacc("TRN2", target_bir_lowering=False, debug=False)

    g6d = nc.dram_tensor('g6', [P, NT * 6], F32, kind="ExternalInput").ap()
    gohd = nc.dram_tensor('goh', [P, NT * 4], F16, kind="ExternalInput").ap()
    mmxd = nc.dram_tensor('mmx', [P, NT * ASPAN * 16], F16, kind="ExternalInput").ap()
    w3d = nc.dram_tensor('w3', [P, 16 * K_MIX], F16, kind="ExternalInput").ap()
    e2d = nc.dram_tensor('e2full', [P, NB * TA], F16, kind="ExternalInput").ap()
    sstrd = nc.dram_tensor('sstr', [P, NB * S_PER_CORE], F16, kind="ExternalInput").ap()
    einitd = nc.dram_tensor('einit', [S_PER_CORE, 1], F32, kind="ExternalInput").ap()
    onesd = nc.dram_tensor('onesc', [P, 1], F16, kind="ExternalInput").ap()
    zd = nc.dram_tensor('zed', [P, NT * 128], F16, kind="ExternalInput").ap()
    eout = nc.dram_tensor('eout', [S_PER_CORE, 1], F32, kind="ExternalOutput").ap()

    V = None  # set below per-engine helpers

    with tile.TileContext(nc) as tc, ExitStack() as ctx:
        cpool = ctx.enter_context(tc.tile_pool(name="const", bufs=1))
        gpool = ctx.enter_context(tc.tile_pool(name="gath", bufs=1))
        tpool = ctx.enter_context(tc.tile_pool(name="temps", bufs=2))
        epool = ctx.enter_context(tc.tile_pool(name="shexp", bufs=cfg['epool_bufs']))
        spool = ctx.enter_context(tc.tile_pool(name="sq", bufs=cfg['spool_bufs']))
        upool = ctx.enter_context(tc.tile_pool(name="upsum", bufs=2, space="PSUM"))
        apool = ctx.enter_context(tc.tile_pool(name="ampsum", bufs=2, space="PSUM"))
        mpool = ctx.enter_context(tc.tile_pool(name="smpsum", bufs=1, space="PSUM"))

        VE, GE, AE = nc.vector, nc.gpsimd, nc.scalar

        # ---- constants ----
        w3_sb = cpool.tile([P, 16 * K_MIX], F16)
        nc.sync.dma_start(w3_sb[:], w3d)
        e2_sb = cpool.tile([P, NB * TA], F16)
        nc.sync.dma_start(e2_sb[:], e2d)
        sstr_sb = cpool.tile([P, NB * S_PER_CORE], F16)
        nc.sync.dma_start(sstr_sb[:], sstrd)
        ones_sb = cpool.tile([P, 1], F16)
        nc.sync.dma_start(ones_sb[:], onesd)
        e_acc = cpool.tile([S_PER_CORE, 1], F32)
        nc.sync.dma_start(e_acc[:], einitd)
        bias_hpi = cpool.tile([P, 1], F32)
        nc.gpsimd.memset(bias_hpi[:], float(np.pi / 2))

        # ---- full-size gather tables / per-edge outputs ----
        g6 = gpool.tile([P, NT, 6], F32)
        goh = gpool.tile([P, NT, 4], F16)
        mmx = gpool.tile([P, NT, ASPAN * 16], F16)
        sh = gpool.tile([P, NT, 16], F16)
        pf = gpool.tile([P, NT, 128], F16)
        nc.gpsimd.memset(sh[:, :, M_OFF[0]:M_OFF[0] + 1], 1.0)
        # one-time zero of pf (pad cols must stay 0); body (repeat) excludes it
        for z0 in range(0, NT, TPB * 4):
            z1 = min(z0 + TPB * 4, NT)
            nc.sync.dma_start(
                pf[:, z0:z1, :],
                zd.rearrange("p (t c) -> p t c", c=128)[:, z0:z1, :])

        if cfg.get('chunk_sizes'):
            chunks = []
            b0 = 0
            for s in cfg['chunk_sizes']:
                chunks.append((b0, min(b0 + s, NB)))
                b0 += s
            chunks = [(a, b_) for (a, b_) in chunks if a < NB]
        elif cfg['nchunks'] == 'grow':
            chunks = _grow_chunks(NB)
        else:
            chunks = _chunks(NB, cfg['nchunks'])
            if cfg.get('first_small'):
                fs = cfg['first_small']
                flat = []
                for (a, b_) in chunks:
                    flat.extend(range(a, b_))
                sizes = [fs]
                rest = NB - fs
                n = cfg['nchunks'] - 1
                bs, rm = divmod(rest, n)
                sizes += [bs + (1 if i < rm else 0) for i in range(n)]
                chunks = []
                b0 = 0
                for s in sizes:
                    chunks.append((b0, b0 + s))
                    b0 += s

        for _rep in range(repeat):
          for (d0, d1) in _chunks(NB, cfg['ndma']):
            u0, u1 = d0 * TPB, d1 * TPB
            nc.sync.dma_start(
                g6[:, u0:u1, :],
                g6d.rearrange("p (t c) -> p t c", c=6)[:, u0:u1, :])
            nc.sync.dma_start(
                goh[:, u0:u1, :],
                gohd.rearrange("p (t c) -> p t c", c=4)[:, u0:u1, :])
            nc.sync.dma_start(
                mmx[:, u0:u1, :],
                mmxd.rearrange("p (t c) -> p t c", c=ASPAN * 16)[:, u0:u1, :])
          for (b0, b1) in chunks:
            t0, t1 = b0 * TPB, b1 * TPB
            T = t1 - t0

            def TT(eng, out, a, b_, op1, s=1.0, op0=OP.mult):
                if eng is GE:
                    # HW GPSIMD has no scalar_tensor_tensor opcode
                    assert s == 1.0 and op0 == OP.mult
                    eng.tensor_tensor(out, a, b_, op1)
                else:
                    eng.scalar_tensor_tensor(out, a, s, b_, op0, op1)

            gs = g6[:, t0:t1, 0:3]
            gr = g6[:, t0:t1, 3:6]
            rvec = tpool.tile([P, T, 3], F32, tag="rvec")
            TT(VE, rvec[:], gr, gs, OP.subtract)
            sq3 = tpool.tile([P, T, 3], F32, tag="sq3")
            e_sq3 = GE if 'sq3' in cfg['geo_pool'] else VE
            TT(e_sq3, sq3[:], rvec[:], rvec[:], OP.mult)
            r2 = tpool.tile([P, T, 1], F32, tag="r2")
            TT(VE, r2[:], sq3[:, :, 0:1], sq3[:, :, 1:2], OP.add)
            TT(VE, r2[:], sq3[:, :, 2:3], r2[:], OP.add, s=1e-12, op0=OP.add)
            # Newton fast-inverse-sqrt (2 iterations)
            ri = tpool.tile([P, T, 1], I32, tag="ri")
            VE.tensor_scalar(ri[:], r2[:].bitcast(I32), 1, None,
                             OP.logical_shift_right)
            VE.tensor_scalar(ri[:], ri[:], -1, MAGIC, OP.mult, OP.add)
            rinv = ri[:].bitcast(F32)
            h_t = tpool.tile([P, T, 1], F32, tag="h_t")
            w_t = tpool.tile([P, T, 1], F32, tag="w_t")
            NE = GE if 'newton' in cfg['geo_pool'] else VE
            for _it in range(2):
                # y' = y*(1.5 - 0.5*r2*y*y), 3 fused instrs
                TT(NE, h_t[:], rinv, rinv, OP.mult)
                TT(NE, w_t[:], h_t[:], r2[:], OP.mult, s=-0.5)
                TT(NE, rinv, w_t[:], rinv, OP.mult, s=1.5, op0=OP.add)
            # xr = min(r2*rinv/CUTOFF, 1); xrp = xr + 1e-3
            xr = tpool.tile([P, T, 1], F32, tag="xr")
            TT(VE, xr[:], r2[:], rinv, OP.mult, s=1.0 / CUTOFF)
            VE.tensor_scalar(xr[:], xr[:], 1.0, None, OP.min)
            xrp = tpool.tile([P, T, 1], F32, tag="xrp")
            VE.tensor_scalar(xrp[:], xr[:], 1e-3, None, OP.add)
            xrinv = tpool.tile([P, T, 1], F32, tag="xrinv")
            VE.reciprocal(xrinv[:], xrp[:])
            # u = rvec * rinv
            u = tpool.tile([P, T, 3], F32, tag="u")
            TT(VE, u[:], rvec[:], rinv.broadcast_to([P, T, 3]), OP.mult)
            fc = tpool.tile([P, T, 1], F32, tag="fc")
            AE.activation(fc[:], xr[:], ACTF.Sin,
                          bias=bias_hpi[:], scale=float(-np.pi))
            sin_t = tpool.tile([P, T, 8], F32, tag="sin_t")
            AE.activation(sin_t[:, :, 0:1], xr[:], ACTF.Sin,
                          scale=float(np.pi))
            stmp = tpool.tile([P, T, 1], F32, tag="stmp")
            CE = GE if 'cheb' in cfg['geo_pool'] else VE
            TT(CE, sin_t[:, :, 1:2], fc[:], sin_t[:, :, 0:1], OP.mult, s=2.0)
            for n in range(3, 9):
                TT(CE, stmp[:], fc[:], sin_t[:, :, n - 2:n - 1], OP.mult, s=2.0)
                TT(CE, sin_t[:, :, n - 1:n], stmp[:], sin_t[:, :, n - 3:n - 2],
                   OP.subtract, s=1.0, op0=OP.mult)
            # wfac = (fc+1)*xrinv ; ohw = goh*wfac
            wfac = tpool.tile([P, T, 1], F32, tag="wfac")
            TT(VE, wfac[:], fc[:], xrinv[:], OP.mult, s=1.0, op0=OP.add)
            ohw = tpool.tile([P, T, 4], F16, tag="ohw")
            TT(VE, ohw[:], goh[:, t0:t1, :], wfac[:].broadcast_to([P, T, 4]),
               OP.mult)
            # pf bands: tile t owns cols [32*(t%4), +32); 4D TT per group
            pfv = pf[:, t0:t1, :].rearrange("p (tq gg) c -> p tq gg c", gg=4)
            ohv = ohw[:].rearrange("p (tq gg) s -> p tq gg s", gg=4)
            siv = sin_t[:].rearrange("p (tq gg) r -> p tq gg r", gg=4)
            TQ = T // 4
            for g_ in range(4):
                e_pf = GE if g_ < cfg.get('pf_pool_n', 2) else VE
                e_pf.tensor_tensor(
                    pfv[:, :, g_, 32 * g_:32 * (g_ + 1)].rearrange(
                        "p tq (s r) -> p tq s r", r=8),
                    ohv[:, :, g_, :].unsqueeze(3).broadcast_to([P, TQ, 4, 8]),
                    siv[:, :, g_, :].unsqueeze(2).broadcast_to([P, TQ, 4, 8]),
                    OP.mult)

            # ---- spherical harmonics ----
            x = u[:, :, 0:1]
            y = u[:, :, 1:2]
            z = u[:, :, 2:3]
            shc = sh[:, t0:t1, :]
            o1, o2 = M_OFF[1], M_OFF[2]
            VE.tensor_scalar(shc[:, :, o1:o1 + 2], u[:, :, 1:3], C1, None, OP.mult)
            VE.tensor_scalar(shc[:, :, o1 + 2:o1 + 3], x, C1, None, OP.mult)
            u2 = tpool.tile([P, T, 3], F32, tag="u2")
            e_u2 = GE if 'u2' in cfg['geo_pool'] else VE
            TT(e_u2, u2[:], u[:], u[:], OP.mult)
            x2 = u2[:, :, 0:1]
            y2 = u2[:, :, 1:2]
            z2 = u2[:, :, 2:3]
            e_pr = GE if 'prods' in cfg['geo_pool'] else VE
            prods = tpool.tile([P, T, 3], F32, tag="prods")  # xy, yz, xz
            TT(e_pr, prods[:, :, 0:2], u[:, :, 0:2], u[:, :, 1:3], OP.mult)
            TT(e_pr, prods[:, :, 2:3], x, z, OP.mult)
            xy = prods[:, :, 0:1]
            yz = prods[:, :, 1:2]
            xz = prods[:, :, 2:3]
            VE.tensor_scalar(shc[:, :, o2:o2 + 2], prods[:, :, 0:2], C2A, None, OP.mult)
            VE.tensor_scalar(shc[:, :, o2 + 2:o2 + 3], z2, 3.0 * C2B, C2B,
                             OP.mult, OP.subtract)
            VE.tensor_scalar(shc[:, :, o2 + 3:o2 + 4], xz, C2A, None, OP.mult)
            xmy = tpool.tile([P, T, 1], F32, tag="xmy")
            TT(e_pr, xmy[:], x2, y2, OP.subtract)
            VE.tensor_scalar(shc[:, :, o2 + 4:o2 + 5], xmy[:], C2C, None, OP.mult)
            # l3
            LE = GE if 'l3' in cfg['geo_pool'] else VE
            t3a = tpool.tile([P, T, 1], F32, tag="t3a")
            TT(LE, t3a[:], x2, y2, OP.subtract, s=3.0)          # 3x2-y2
            TT(LE, shc[:, :, 0:1], t3a[:], y, OP.mult, s=C3A)
            TT(LE, shc[:, :, 1:2], xy, z, OP.mult, s=C3B)
            t511 = tpool.tile([P, T, 1], F32, tag="t511")
            LE.tensor_scalar(t511[:], z2, 5.0 * C3C, C3C, OP.mult, OP.subtract)
            TT(LE, shc[:, :, 2:3], t511[:], y, OP.mult)
            t533 = tpool.tile([P, T, 1], F32, tag="t533")
            LE.tensor_scalar(t533[:], z2, 5.0 * C3D, 3.0 * C3D, OP.mult, OP.subtract)
            TT(LE, shc[:, :, 3:4], t533[:], z, OP.mult)
            TT(LE, shc[:, :, 4:5], t511[:], x, OP.mult)
            TT(LE, shc[:, :, 5:6], xmy[:], z, OP.mult, s=C3B2)
            t3b = tpool.tile([P, T, 1], F32, tag="t3b")
            TT(LE, t3b[:], y2, x2, OP.subtract, s=3.0)   # 3y2 - x2
            TT(LE, shc[:, :, 6:7], t3b[:], x, OP.mult, s=-C3A)

            # ---- per-block phase 1 + phase 2 ----
            for b in range(b0, b1):
                sh_exp = epool.tile([P, TPB, ASPAN, 16], F16, tag="shexp")
                e_se = GE if (cfg['shexp_pool_mod'] and
                              b % cfg['shexp_pool_mod'] == 0) else VE
                e_se.tensor_tensor(
                   sh_exp[:],
                   sh[:, b * TPB:(b + 1) * TPB, :].unsqueeze(2)
                     .broadcast_to([P, TPB, ASPAN, 16]),
                   mmx[:, b * TPB:(b + 1) * TPB, :].rearrange(
                       "p t (a m) -> p t a m", m=16),
                   OP.mult)
                u_ps = upool.tile([P, CPG, ASPAN * 16], F32, tag="ups",
                                  space="PSUM")
                for cc_ in range(CPG):
                    for g in range(NGRP):
                        tl = cc_ * NGRP + g
                        nc.tensor.matmul(
                            u_ps[:, cc_, :],
                            lhsT=pf[:, b * TPB + tl, :],
                            rhs=sh_exp[:, tl, :, :].rearrange(
                                "p a m -> p (a m)"),
                            start=(g == 0), stop=(g == NGRP - 1))
                u_sb = epool.tile([P, CPG * ASPAN * 16], F16, tag="usb")
                AE.copy(u_sb[:], u_ps[:].rearrange("p c am -> p (c am)"))

                uv = u_sb[:].rearrange("q (ca m) -> q ca m", m=16)
                sq = spool.tile([P, TA, 16], F16, tag="sq")
                CA = CPG * ASPAN  # 30
                t4ab = spool.tile([P, 2, TA, 4], F16, tag="t4ab")
                for pair_i, (la, lb) in enumerate(((3, 0), (1, 2))):
                    am = apool.tile([P, NGRP, 256], F32, tag="am",
                                    space="PSUM")
                    amoff = 0
                    for l in (la, lb):
                        ml = M_LEN[l]
                        for g in range(NGRP):
                            nc.tensor.matmul(
                                am[:, g, amoff:amoff + CA * ml],
                                lhsT=w3_sb[:, (l * NGRP + g) * K_MIX:
                                           (l * NGRP + g + 1) * K_MIX],
                                rhs=uv[:, :, M_OFF[l]:M_OFF[l] + ml],
                                start=True, stop=True)
                        # squares: all 4 g at once -> sq[(g,c,a), moff:+ml]
                        # (3D APs; same element order as the 4D view)
                        dst = sq[:, :, M_OFF[l]:M_OFF[l] + ml]
                        src = am[:, :, amoff:amoff + CA * ml]
                        if f'l{l}' in cfg['sq_dve']:
                            VE.tensor_tensor(dst, src, src, OP.mult)
                        else:
                            AE.activation(dst, src, ACTF.Square)
                        amoff += CA * ml
                    # partial reduce of this pair's 8 m-cols: 8 -> 4
                    mo = pair_i * 8
                    VE.tensor_tensor(t4ab[:, pair_i, :, :],
                                     sq[:, :, mo:mo + 4],
                                     sq[:, :, mo + 4:mo + 8], OP.add)
                # combine pairs: 4+4 -> 4 -> 2 -> 1 (fp16 TT, 2x)
                t4 = spool.tile([P, TA, 4], F16, tag="t4")
                VE.tensor_tensor(t4[:], t4ab[:, 0, :, :], t4ab[:, 1, :, :],
                                 OP.add)
                t2 = spool.tile([P, TA, 2], F16, tag="t2")
                VE.tensor_tensor(t2[:], t4[:, :, 0:2], t4[:, :, 2:4], OP.add)
                Bt = spool.tile([P, TA], F16, tag="B")
                VE.tensor_tensor(Bt[:].unsqueeze(2), t2[:, :, 0:1],
                                 t2[:, :, 1:2], OP.add)
                e_b4 = VE if cfg['b4h_eng'] == 'dve' else GE
                B4 = spool.tile([P, TA], F16, tag="B4")
                e_b4.tensor_tensor(B4[:], Bt[:], Bt[:], OP.mult)
                H = spool.tile([P, TA], F16, tag="H")
                e_b4.tensor_tensor(H[:], B4[:], e2_sb[:, b * TA:(b + 1) * TA],
                                   OP.mult)
                at_ps = mpool.tile([TA, 1], F32, tag="at", space="PSUM")
                nc.tensor.matmul(at_ps[:], lhsT=H[:], rhs=ones_sb[:],
                                 start=True, stop=True)
                at_sb = spool.tile([TA, 1], F16, tag="atsb")
                if cfg.get('atsb_dve'):
                    VE.tensor_copy(at_sb[:], at_ps[:])
                else:
                    AE.copy(at_sb[:], at_ps[:])
                eb_ps = mpool.tile([S_PER_CORE, 1], F32, tag="eb", space="PSUM")
                nc.tensor.matmul(
                    eb_ps[:],
                    lhsT=sstr_sb[0:TA, b * S_PER_CORE:(b + 1) * S_PER_CORE],
                    rhs=at_sb[:], start=True, stop=True)
                VE.scalar_tensor_tensor(e_acc[:], e_acc[:], 1.0, eb_ps[:],
                                        OP.mult, OP.add)

        nc.sync.dma_start(eout, e_acc[:])

    nc.compile()
    return nc


_CACHE = {}


def _get_program(NT, NB):
    key = (NT, NB)
    if key not in _CACHE:
        _CACHE[key] = build_program(NT, NB)
    return _CACHE[key]


def run(inputs, trace=False, **kwargs):
    in_maps, NT, NB = _preprocess(inputs)
    nc = _get_program(NT, NB)
    res = run_bass_kernel_spmd(nc, in_maps, core_ids=list(range(N_CORES)),
                               trace=trace, **kwargs)
    out = np.concatenate([res.results[c]['eout'][:, 0] for c in range(N_CORES)])
    return out.astype(np.float32), res


def kernel(**inputs):
    out, _ = run(inputs)
    return out


# revision 11
# speedup vs baseline: 1.8761x; 1.8761x over previous
"""Trainium2 Bass kernel for nn_Model_1245540515968 (gnn_message_passing), v2.

Self-contained: kernel(**inputs) -> np.ndarray [128] per-structure energies.

Strategy (8 cores, graph/data parallel, same algebra as v1 but restructured):
  - Shard by structure: core c owns structures [16c,16c+16) and their atoms.
  - Atoms packed 6-per-tile (128 edge slots); ASPAN=6, TPB=20 tiles/block
    -> 120 atom slots per block, NB blocks.
  - fp16 matmul operands everywhere (PE 1 cyc/row vs fp32's 4).
  - Host-expanded mask mm_exp [P,NT,6,16] fp16 so sh_exp = STT(sh, mm) hits
    the DVE 4x_2p mode (0.26 ns/elem).
  - All elementwise via scalar_tensor_tensor / tensor_scalar (TensorScalarPtr
    -> 2x_2p/4x_2p modes) instead of tensor_tensor (2x_1p only).
  - Newton fast-inverse-sqrt on DVE so Act only needs {sin, square, copy}
    (single act table: trig_and_small; no 1283ns table switches).
  - Phase-1 U matmuls write 4 partition-groups of PSUM (32 rows each), so
    the PSUM->SBUF U copy is [128,480] (484 free elems) not [32,1920].
  - Phase 2: Am = w3_l^T U per (l, g-pair); Act Square PSUM->SBUF fp16;
    DVE pairwise-add tree over m (4x mode); B4/H fp16; per-block energy via
    two small matmuls; fp32 accumulation of per-structure energies.
"""
import os
import sys
from contextlib import ExitStack

import numpy as np

for _p in ("/opt/trn_rl_repo",):
    if _p not in sys.path and os.path.isdir(_p):
        sys.path.insert(0, _p)

import concourse.bass as bass
import concourse.tile as tile
from concourse import bacc, mybir
from concourse.bass_utils import run_bass_kernel_spmd

F32 = mybir.dt.float32
F16 = mybir.dt.float16
I32 = mybir.dt.int32
AX = mybir.AxisListType
OP = mybir.AluOpType
ACTF = mybir.ActivationFunctionType

N_ATOMS = 10000
N_EDGES = 200000
N_SPECIES = 4
N_RAD = 8
N_MAX = [8, 6, 4, 2]
K_MIX = 128
N_STRUCT = 128
CUTOFF = 5.0
N_CORES = 8
S_PER_CORE = N_STRUCT // N_CORES
P = 128
ASPAN = 6           # atoms (lanes) per tile
TPB = 20            # tiles per block
TA = ASPAN * TPB    # atom slots per block = 120
NGRP = 4            # psum partition groups (tl % 4)
CPG = TPB // NGRP   # tiles per group = 5

# sh_full column order: [sh3 (7), l0-const (1), sh1 (3), sh2 (5)]
M_OFF = {3: 0, 0: 7, 1: 8, 2: 11}
M_LEN = {0: 1, 1: 3, 2: 5, 3: 7}

C1 = 0.4886025119029199
C2A = 1.0925484305920792
C2B = 0.31539156525252005
C2C = 0.5462742152960396
C3A = 0.5900435899266435
C3B = 2.890611442640554
C3B2 = 1.445305721320277
C3C = 0.4570457994644658
C3D = 0.3731763325901154
L0C = 0.28209479177387814
MAGIC = 0x5F3759DF


# ----------------------------------------------------------------------------
# Host preprocessing
# ----------------------------------------------------------------------------

def _pack_tiles(atom_ids, degs, nbins):
    """Worst-fit decreasing into a fixed number of bins, each <=ASPAN atoms
    and <=P edge slots. Returns list of tiles (atom-id lists) or None."""
    import heapq
    order = np.argsort(-degs, kind='stable')
    heap = [(-P, j) for j in range(nbins)]
    heapq.heapify(heap)
    used = [0] * nbins
    tiles = [[] for _ in range(nbins)]
    for i in order:
        a, d = int(atom_ids[i]), int(degs[i])
        tmp = []
        placed = False
        while heap:
            negfree, j = heapq.heappop(heap)
            if len(tiles[j]) < ASPAN and used[j] + d <= P:
                used[j] += d
                tiles[j].append(a)
                if len(tiles[j]) < ASPAN:
                    heapq.heappush(heap, (-(P - used[j]), j))
                placed = True
                break
            tmp.append((negfree, j))
            if -negfree < d:
                break
        for it in tmp:
            heapq.heappush(heap, it)
        if not placed:
            return None
    return tiles


def _preprocess(inputs):
    species = np.asarray(inputs['species'])
    senders = np.asarray(inputs['senders'])
    receivers = np.asarray(inputs['receivers'])
    batch_seg = np.asarray(inputs['batch_seg'])
    positions = np.asarray(inputs['positions'], dtype=np.float32)

    struct_starts = np.searchsorted(batch_seg, np.arange(N_STRUCT + 1))
    core_hi = struct_starts[(np.arange(N_CORES) + 1) * S_PER_CORE]
    core_lo = np.concatenate([[0], core_hi[:-1]])

    # receiver-sorted edge index
    r_order = np.argsort(receivers, kind='stable')
    r_sorted = receivers[r_order]
    deg = np.bincount(receivers, minlength=N_ATOMS)
    estart = np.concatenate([[0], np.cumsum(deg)])

    amax = max(int(core_hi[c] - core_lo[c]) for c in range(N_CORES))
    NT = -(-(-(-amax // ASPAN)) // TPB) * TPB
    while True:
        cores = []
        for c in range(N_CORES):
            a_lo, a_hi = int(core_lo[c]), int(core_hi[c])
            aids = np.arange(a_lo, a_hi)
            tiles = _pack_tiles(aids, deg[a_lo:a_hi], NT)
            if tiles is None:
                cores = None
                break
            cores.append(dict(a_lo=a_lo, a_hi=a_hi, tiles=tiles))
        if cores is not None:
            break
        NT += TPB
    NB = NT // TPB

    # weight transforms
    emb = np.asarray(inputs['emb'], np.float32)
    emb2 = np.asarray(inputs['emb2'], np.float32)
    w_out = np.asarray(inputs['w_out'], np.float32)
    scal = float(np.asarray(inputs['scaling'])[0])
    W3 = np.zeros((32, 4 * K_MIX), np.float32)
    for l in range(4):
        w_rad = np.asarray(inputs[f'w_rad{l}'], np.float32) * 0.5  # fcut 0.5
        w_mix = np.asarray(inputs[f'w_mix{l}'], np.float32)
        n_l = N_MAX[l]
        W2 = np.einsum('sc,ri->sric', emb, w_rad).reshape(32, n_l * 16)
        w3 = (W2 @ w_mix) * (2 * l + 1) ** -0.25
        if l == 0:
            w3 = w3 * L0C
        W3[:, l * K_MIX:(l + 1) * K_MIX] = w3
    E2s = (emb2 * w_out[None, :] * scal).astype(np.float32)     # [4, 128]
    cw = np.asarray(inputs['comp_weights'], np.float32)
    cw_struct = np.zeros(N_STRUCT, np.float32)
    np.add.at(cw_struct, batch_seg, cw[species])

    oh_tab = (species[:, None] == np.arange(N_SPECIES)[None, :]).astype(np.float16)

    w3q = np.zeros((P, 16 * K_MIX), np.float32)
    for l in range(4):
        for g in range(NGRP):
            w3q[32 * g:32 * (g + 1), (l * NGRP + g) * K_MIX:
                (l * NGRP + g + 1) * K_MIX] = W3[:, l * K_MIX:(l + 1) * K_MIX]
    shared = dict(w3=np.ascontiguousarray(w3q).astype(np.float16))
    in_maps = []
    for ci, c in enumerate(cores):
        g6 = np.zeros((NT, P, 6), np.float32)
        goh = np.zeros((NT, P, 4), np.float16)
        mm = np.zeros((NT, P, ASPAN), np.float16)
        slot_atom = -np.ones((NB, TA), np.int64)   # [block, ta] -> atom
        for t, tile_atoms in enumerate(c['tiles']):
            b, tl = t // TPB, t % TPB
            g, cc_ = tl % NGRP, tl // NGRP
            s = 0
            for a_local, a in enumerate(tile_atoms):
                d = int(deg[a])
                eds = r_order[estart[a]:estart[a] + d]
                g6[t, s:s + d, 0:3] = positions[senders[eds]]
                g6[t, s:s + d, 3:6] = positions[receivers[eds]]
                goh[t, s:s + d, :] = oh_tab[senders[eds]]
                mm[t, s:s + d, a_local] = 1.0
                ta = g * (CPG * ASPAN) + cc_ * ASPAN + a_local
                slot_atom[b, ta] = a
                s += d
        mm_exp = np.broadcast_to(mm[:, :, :, None], (NT, P, ASPAN, 16))
        m = dict(shared)
        m['g6'] = np.ascontiguousarray(g6.transpose(1, 0, 2).reshape(P, NT * 6))
        m['goh'] = np.ascontiguousarray(goh.transpose(1, 0, 2).reshape(P, NT * 4))
        m['mmx'] = np.ascontiguousarray(
            mm_exp.transpose(1, 0, 2, 3).reshape(P, NT * ASPAN * 16))
        sa = slot_atom.reshape(-1)
        valid = sa >= 0
        sp_slot = np.where(valid, species[np.clip(sa, 0, None)], 0)
        e2full = np.where(valid[None, :], E2s.T[:, sp_slot], 0.0)
        m['e2full'] = np.ascontiguousarray(e2full).astype(np.float16)  # [128, NB*TA]
        S = np.zeros((NB, TA, S_PER_CORE), np.float16)
        bidx = np.nonzero(valid)[0]
        S[bidx // TA, bidx % TA,
          batch_seg[sa[valid]] - ci * S_PER_CORE] = 1.0
        sfull = np.zeros((P, NB * S_PER_CORE), np.float16)
        sfull[:TA, :] = S.transpose(1, 0, 2).reshape(TA, NB * S_PER_CORE)
        m['sstr'] = np.ascontiguousarray(sfull)
        m['einit'] = cw_struct[ci * S_PER_CORE:(ci + 1) * S_PER_CORE].reshape(
            S_PER_CORE, 1).copy()
        m['onesc'] = np.ones((P, 1), np.float16)
        m['zed'] = np.zeros((P, NT * 128), np.float16)
        in_maps.append(m)
    return in_maps, NT, NB


# ----------------------------------------------------------------------------
# Bass program
# ----------------------------------------------------------------------------

def _chunks(NB, n):
    n = min(n, NB)
    base, rem = divmod(NB, n)
    out, b0 = [], 0
    for i in range(n):
        nb = base + (1 if i < rem else 0)
        out.append((b0, b0 + nb))
        b0 += nb
    return out


def _grow_chunks(NB, first=1, mult=2):
    """Geometrically growing chunk sizes: [1, 2, 4, 4, ...] summing to NB."""
    out, b0, sz = [], 0, first
    while b0 < NB:
        nb = min(sz, NB - b0)
        out.append((b0, b0 + nb))
        b0 += nb
        sz = min(sz * mult, 4)
    return out


CFG = dict(nchunks=2, ndma=4, sq_dve=(), geo_pool=('sq3', 'u2', 'prods'),
           pf_eng='pool', b4h_eng='pool', shexp_pool_mod=5)


def build_program(NT, NB, repeat=1):
    cfg = CFG
    nc = bacc.Bacc("TRN2", target_bir_lowering=False, debug=False)

    g6d = nc.dram_tensor('g6', [P, NT * 6], F32, kind="ExternalInput").ap()
    gohd = nc.dram_tensor('goh', [P, NT * 4], F16, kind="ExternalInput").ap()
    mmxd = nc.dram_tensor('mmx', [P, NT * ASPAN * 16], F16, kind="ExternalInput").ap()
    w3d = nc.dram_tensor('w3', [P, 16 * K_MIX], F16, kind="ExternalInput").ap()
    e2d = nc.dram_tensor('e2full', [P, NB * TA], F16, kind="ExternalInput").ap()
    sstrd = nc.dram_tensor('sstr', [P, NB * S_PER_CORE], F16, kind="ExternalInput").ap()
    einitd = nc.dram_tensor('einit', [S_PER_CORE, 1], F32, kind="ExternalInput").ap()
    onesd = nc.dram_tensor('onesc', [P, 1], F16, kind="ExternalInput").ap()
    zd = nc.dram_tensor('zed', [P, NT * 128], F16, kind="ExternalInput").ap()
    eout = nc.dram_tensor('eout', [S_PER_CORE, 1], F32, kind="ExternalOutput").ap()

    V = None  # set below per-engine helpers

    with tile.TileContext(nc) as tc, ExitStack() as ctx:
        cpool = ctx.enter_context(tc.tile_pool(name="const", bufs=1))
        gpool = ctx.enter_context(tc.tile_pool(name="gath", bufs=1))
        tpool = ctx.enter_context(tc.tile_pool(name="temps", bufs=2))
        epool = ctx.enter_context(tc.tile_pool(name="shexp", bufs=cfg['epool_bufs']))
        spool = ctx.enter_context(tc.tile_pool(name="sq", bufs=cfg['spool_bufs']))
        upool = ctx.enter_context(tc.tile_pool(name="upsum", bufs=2, space="PSUM"))
        apool = ctx.enter_context(tc.tile_pool(name="ampsum", bufs=2, space="PSUM"))
        mpool = ctx.enter_context(tc.tile_pool(name="smpsum", bufs=1, space="PSUM"))

        VE, GE, AE = nc.vector, nc.gpsimd, nc.scalar

        # ---- constants ----
        w3_sb = cpool.tile([P, 16 * K_MIX], F16)
        nc.sync.dma_start(w3_sb[:], w3d)
        e2_sb = cpool.tile([P, NB * TA], F16)
        nc.sync.dma_start(e2_sb[:], e2d)
        sstr_sb = cpool.tile([P, NB * S_PER_CORE], F16)
        nc.sync.dma_start(sstr_sb[:], sstrd)
        ones_sb = cpool.tile([P, 1], F16)
        nc.sync.dma_start(ones_sb[:], onesd)
        e_acc = cpool.tile([S_PER_CORE, 1], F32)
        nc.sync.dma_start(e_acc[:], einitd)
        bias_hpi = cpool.tile([P, 1], F32)
        nc.gpsimd.memset(bias_hpi[:], float(np.pi / 2))

        # ---- full-size gather tables / per-edge outputs ----
        g6 = gpool.tile([P, NT, 6], F32)
        goh = gpool.tile([P, NT, 4], F16)
        mmx = gpool.tile([P, NT, ASPAN * 16], F16)
        sh = gpool.tile([P, NT, 16], F16)
        pf = gpool.tile([P, NT, 128], F16)
        nc.gpsimd.memset(sh[:, :, M_OFF[0]:M_OFF[0] + 1], 1.0)
        # one-time zero of pf (pad cols must stay 0); body (repeat) excludes it
        for z0 in range(0, NT, TPB * 4):
            z1 = min(z0 + TPB * 4, NT)
            nc.sync.dma_start(
                pf[:, z0:z1, :],
                zd.rearrange("p (t c) -> p t c", c=128)[:, z0:z1, :])

        if cfg.get('chunk_sizes'):
            chunks = []
            b0 = 0
            for s in cfg['chunk_sizes']:
                chunks.append((b0, min(b0 + s, NB)))
                b0 += s
            chunks = [(a, b_) for (a, b_) in chunks if a < NB]
        elif cfg['nchunks'] == 'grow':
            chunks = _grow_chunks(NB)
        else:
            chunks = _chunks(NB, cfg['nchunks'])
            if cfg.get('first_small'):
                fs = cfg['first_small']
                flat = []
                for (a, b_) in chunks:
                    flat.extend(range(a, b_))
                sizes = [fs]
                rest = NB - fs
                n = cfg['nchunks'] - 1
                bs, rm = divmod(rest, n)
                sizes += [bs + (1 if i < rm else 0) for i in range(n)]
                chunks = []
                b0 = 0
                for s in sizes:
                    chunks.append((b0, b0 + s))
                    b0 += s

        for _rep in range(repeat):
          for (d0, d1) in _chunks(NB, cfg['ndma']):
            u0, u1 = d0 * TPB, d1 * TPB
            nc.sync.dma_start(
                g6[:, u0:u1, :],
                g6d.rearrange("p (t c) -> p t c", c=6)[:, u0:u1, :])
            nc.sync.dma_start(
                goh[:, u0:u1, :],
                gohd.rearrange("p (t c) -> p t c", c=4)[:, u0:u1, :])
            nc.sync.dma_start(
                mmx[:, u0:u1, :],
                mmxd.rearrange("p (t c) -> p t c", c=ASPAN * 16)[:, u0:u1, :])
          for (b0, b1) in chunks:
            t0, t1 = b0 * TPB, b1 * TPB
            T = t1 - t0

            def TT(eng, out, a, b_, op1, s=1.0, op0=OP.mult):
                if eng is GE:
                    # HW GPSIMD has no scalar_tensor_tensor opcode
                    assert s == 1.0 and op0 == OP.mult
                    eng.tensor_tensor(out, a, b_, op1)
                else:
                    eng.scalar_tensor_tensor(out, a, s, b_, op0, op1)

            gs = g6[:, t0:t1, 0:3]
            gr = g6[:, t0:t1, 3:6]
            rvec = tpool.tile([P, T, 3], F32, tag="rvec")
            TT(VE, rvec[:], gr, gs, OP.subtract)
            sq3 = tpool.tile([P, T, 3], F32, tag="sq3")
            e_sq3 = GE if 'sq3' in cfg['geo_pool'] else VE
            TT(e_sq3, sq3[:], rvec[:], rvec[:], OP.mult)
            r2 = tpool.tile([P, T, 1], F32, tag="r2")
            TT(VE, r2[:], sq3[:, :, 0:1], sq3[:, :, 1:2], OP.add)
            TT(VE, r2[:], sq3[:, :, 2:3], r2[:], OP.add, s=1e-12, op0=OP.add)
            # Newton fast-inverse-sqrt (2 iterations)
            ri = tpool.tile([P, T, 1], I32, tag="ri")
            VE.tensor_scalar(ri[:], r2[:].bitcast(I32), 1, None,
                             OP.logical_shift_right)
            VE.tensor_scalar(ri[:], ri[:], -1, MAGIC, OP.mult, OP.add)
            rinv = ri[:].bitcast(F32)
            h_t = tpool.tile([P, T, 1], F32, tag="h_t")
            w_t = tpool.tile([P, T, 1], F32, tag="w_t")
            NE = GE if 'newton' in cfg['geo_pool'] else VE
            for _it in range(2):
                # y' = y*(1.5 - 0.5*r2*y*y), 3 fused instrs
                TT(NE, h_t[:], rinv, rinv, OP.mult)
                TT(NE, w_t[:], h_t[:], r2[:], OP.mult, s=-0.5)
                TT(NE, rinv, w_t[:], rinv, OP.mult, s=1.5, op0=OP.add)
            # xr = min(r2*rinv/CUTOFF, 1); xrp = xr + 1e-3
            xr = tpool.tile([P, T, 1], F32, tag="xr")
            TT(VE, xr[:], r2[:], rinv, OP.mult, s=1.0 / CUTOFF)
            VE.tensor_scalar(xr[:], xr[:], 1.0, None, OP.min)
            xrp = tpool.tile([P, T, 1], F32, tag="xrp")
            VE.tensor_scalar(xrp[:], xr[:], 1e-3, None, OP.add)
            xrinv = tpool.tile([P, T, 1], F32, tag="xrinv")
            VE.reciprocal(xrinv[:], xrp[:])
            # u = rvec * rinv
            u = tpool.tile([P, T, 3], F32, tag="u")
            TT(VE, u[:], rvec[:], rinv.broadcast_to([P, T, 3]), OP.mult)
            fc = tpool.tile([P, T, 1], F32, tag="fc")
            AE.activation(fc[:], xr[:], ACTF.Sin,
                          bias=bias_hpi[:], scale=float(-np.pi))
            sin_t = tpool.tile([P, T, 8], F32, tag="sin_t")
            AE.activation(sin_t[:, :, 0:1], xr[:], ACTF.Sin,
                          scale=float(np.pi))
            stmp = tpool.tile([P, T, 1], F32, tag="stmp")
            CE = GE if 'cheb' in cfg['geo_pool'] else VE
            TT(CE, sin_t[:, :, 1:2], fc[:], sin_t[:, :, 0:1], OP.mult, s=2.0)
            for n in range(3, 9):
                TT(CE, stmp[:], fc[:], sin_t[:, :, n - 2:n - 1], OP.mult, s=2.0)
                TT(CE, sin_t[:, :, n - 1:n], stmp[:], sin_t[:, :, n - 3:n - 2],
                   OP.subtract, s=1.0, op0=OP.mult)
            # wfac = (fc+1)*xrinv ; ohw = goh*wfac
            wfac = tpool.tile([P, T, 1], F32, tag="wfac")
            TT(VE, wfac[:], fc[:], xrinv[:], OP.mult, s=1.0, op0=OP.add)
            ohw = tpool.tile([P, T, 4], F16, tag="ohw")
            TT(VE, ohw[:], goh[:, t0:t1, :], wfac[:].broadcast_to([P, T, 4]),
               OP.mult)
            # pf bands: tile t owns cols [32*(t%4), +32); 4D TT per group
            pfv = pf[:, t0:t1, :].rearrange("p (tq gg) c -> p tq gg c", gg=4)
            ohv = ohw[:].rearrange("p (tq gg) s -> p tq gg s", gg=4)
            siv = sin_t[:].rearrange("p (tq gg) r -> p tq gg r", gg=4)
            TQ = T // 4
            for g_ in range(4):
                e_pf = GE if g_ < cfg.get('pf_pool_n', 2) else VE
                e_pf.tensor_tensor(
                    pfv[:, :, g_, 32 * g_:32 * (g_ + 1)].rearrange(
                        "p tq (s r) -> p tq s r", r=8),
                    ohv[:, :, g_, :].unsqueeze(3).broadcast_to([P, TQ, 4, 8]),
                    siv[:, :, g_, :].unsqueeze(2).broadcast_to([P, TQ, 4, 8]),
                    OP.mult)

            # ---- spherical harmonics ----
            x = u[:, :, 0:1]
            y = u[:, :, 1:2]
            z = u[:, :, 2:3]
            shc = sh[:, t0:t1, :]
            o1, o2 = M_OFF[1], M_OFF[2]
            VE.tensor_scalar(shc[:, :, o1:o1 + 2], u[:, :, 1:3], C1, None, OP.mult)
            VE.tensor_scalar(shc[:, :, o1 + 2:o1 + 3], x, C1, None, OP.mult)
            u2 = tpool.tile([P, T, 3], F32, tag="u2")
            e_u2 = GE if 'u2' in cfg['geo_pool'] else VE
            TT(e_u2, u2[:], u[:], u[:], OP.mult)
            x2 = u2[:, :, 0:1]
            y2 = u2[:, :, 1:2]
            z2 = u2[:, :, 2:3]
            e_pr = GE if 'prods' in cfg['geo_pool'] else VE
            prods = tpool.tile([P, T, 3], F32, tag="prods")  # xy, yz, xz
            TT(e_pr, prods[:, :, 0:2], u[:, :, 0:2], u[:, :, 1:3], OP.mult)
            TT(e_pr, prods[:, :, 2:3], x, z, OP.mult)
            xy = prods[:, :, 0:1]
            yz = prods[:, :, 1:2]
            xz = prods[:, :, 2:3]
            VE.tensor_scalar(shc[:, :, o2:o2 + 2], prods[:, :, 0:2], C2A, None, OP.mult)
            VE.tensor_scalar(shc[:, :, o2 + 2:o2 + 3], z2, 3.0 * C2B, C2B,
                             OP.mult, OP.subtract)
            VE.tensor_scalar(shc[:, :, o2 + 3:o2 + 4], xz, C2A, None, OP.mult)
            xmy = tpool.tile([P, T, 1], F32, tag="xmy")
            TT(e_pr, xmy[:], x2, y2, OP.subtract)
            VE.tensor_scalar(shc[:, :, o2 + 4:o2 + 5], xmy[:], C2C, None, OP.mult)
            # l3
            LE = GE if 'l3' in cfg['geo_pool'] else VE
            t3a = tpool.tile([P, T, 1], F32, tag="t3a")
            TT(LE, t3a[:], x2, y2, OP.subtract, s=3.0)          # 3x2-y2
            TT(LE, shc[:, :, 0:1], t3a[:], y, OP.mult, s=C3A)
            TT(LE, shc[:, :, 1:2], xy, z, OP.mult, s=C3B)
            t511 = tpool.tile([P, T, 1], F32, tag="t511")
            LE.tensor_scalar(t511[:], z2, 5.0 * C3C, C3C, OP.mult, OP.subtract)
            TT(LE, shc[:, :, 2:3], t511[:], y, OP.mult)
            t533 = tpool.tile([P, T, 1], F32, tag="t533")
            LE.tensor_scalar(t533[:], z2, 5.0 * C3D, 3.0 * C3D, OP.mult, OP.subtract)
            TT(LE, shc[:, :, 3:4], t533[:], z, OP.mult)
            TT(LE, shc[:, :, 4:5], t511[:], x, OP.mult)
            TT(LE, shc[:, :, 5:6], xmy[:], z, OP.mult, s=C3B2)
            t3b = tpool.tile([P, T, 1], F32, tag="t3b")
            TT(LE, t3b[:], y2, x2, OP.subtract, s=3.0)   # 3y2 - x2
            TT(LE, shc[:, :, 6:7], t3b[:], x, OP.mult, s=-C3A)

            # ---- per-block phase 1 + phase 2 ----
            for b in range(b0, b1):
                sh_exp = epool.tile([P, TPB, ASPAN, 16], F16, tag="shexp")
                e_se = GE if (cfg['shexp_pool_mod'] and
                              b % cfg['shexp_pool_mod'] == 0) else VE
                e_se.tensor_tensor(
                   sh_exp[:],
                   sh[:, b * TPB:(b + 1) * TPB, :].unsqueeze(2)
                     .broadcast_to([P, TPB, ASPAN, 16]),
                   mmx[:, b * TPB:(b + 1) * TPB, :].rearrange(
                       "p t (a m) -> p t a m", m=16),
                   OP.mult)
                u_ps = upool.tile([P, CPG, ASPAN * 16], F32, tag="ups",
                                  space="PSUM")
                for cc_ in range(CPG):
                    for g in range(NGRP):
                        tl = cc_ * NGRP + g
                        nc.tensor.matmul(
                            u_ps[:, cc_, :],
                            lhsT=pf[:, b * TPB + tl, :],
                            rhs=sh_exp[:, tl, :, :].rearrange(
                                "p a m -> p (a m)"),
                            start=(g == 0), stop=(g == NGRP - 1))
                u_sb = epool.tile([P, CPG * ASPAN * 16], F16, tag="usb")
                AE.copy(u_sb[:], u_ps[:].rearrange("p c am -> p (c am)"))

                uv = u_sb[:].rearrange("q (ca m) -> q ca m", m=16)
                sq = spool.tile([P, TA, 16], F16, tag="sq")
                CA = CPG * ASPAN  # 30
                t4ab = spool.tile([P, 2, TA, 4], F16, tag="t4ab")
                for pair_i, (la, lb) in enumerate(((3, 0), (1, 2))):
                    am = apool.tile([P, NGRP, 256], F32, tag="am",
                                    space="PSUM")
                    amoff = 0
                    for l in (la, lb):
                        ml = M_LEN[l]
                        for g in range(NGRP):
                            nc.tensor.matmul(
                                am[:, g, amoff:amoff + CA * ml],
                                lhsT=w3_sb[:, (l * NGRP + g) * K_MIX:
                                           (l * NGRP + g + 1) * K_MIX],
                                rhs=uv[:, :, M_OFF[l]:M_OFF[l] + ml],
                                start=True, stop=True)
                        # squares: all 4 g at once -> sq[(g,c,a), moff:+ml]
                        # (3D APs; same element order as the 4D view)
                        dst = sq[:, :, M_OFF[l]:M_OFF[l] + ml]
                        src = am[:, :, amoff:amoff + CA * ml]
                        if f'l{l}' in cfg['sq_dve']:
                            VE.tensor_tensor(dst, src, src, OP.mult)
                        else:
                            AE.activation(dst, src, ACTF.Square)
                        amoff += CA * ml
                    # partial reduce of this pair's 8 m-cols: 8 -> 4
                    mo = pair_i * 8
                    VE.tensor_tensor(t4ab[:, pair_i, :, :],
                                     sq[:, :, mo:mo + 4],
                                     sq[:, :, mo + 4:mo + 8], OP.add)
                # combine pairs: 4+4 -> 4 -> 2 -> 1 (fp16 TT, 2x)
                t4 = spool.tile([P, TA, 4], F16, tag="t4")
                VE.tensor_tensor(t4[:], t4ab[:, 0, :, :], t4ab[:, 1, :, :],
                                 OP.add)
                t2 = spool.tile([P, TA, 2], F16, tag="t2")
                VE.tensor_tensor(t2[:], t4[:, :, 0:2], t4[:, :, 2:4], OP.add)
                Bt = spool.tile([P, TA], F16, tag="B")
                VE.tensor_tensor(Bt[:].unsqueeze(2), t2[:, :, 0:1],
                                 t2[:, :, 1:2], OP.add)
                e_b4 = VE if cfg['b4h_eng'] == 'dve' else GE
                B4 = spool.tile([P, TA], F16, tag="B4")
                e_b4.tensor_tensor(B4[:], Bt[:], Bt[:], OP.mult)
                H = spool.tile([P, TA], F16, tag="H")
                e_b4.tensor_tensor(H[:], B4[:], e2_sb[:, b * TA:(b + 1) * TA],
                                   OP.mult)
                at_ps = mpool.tile([TA, 1], F32, tag="at", space="PSUM")
                nc.tensor.matmul(at_ps[:], lhsT=H[:], rhs=ones_sb[:],
                                 start=True, stop=True)
                at_sb = spool.tile([TA, 1], F16, tag="atsb")
                if cfg.get('atsb_dve'):
                    VE.tensor_copy(at_sb[:], at_ps[:])
                else:
                    AE.copy(at_sb[:], at_ps[:])
                eb_ps = mpool.tile([S_PER_CORE, 1], F32, tag="eb", space="PSUM")
                nc.tensor.matmul(
                    eb_ps[:],
                    lhsT=sstr_sb[0:TA, b * S_PER_CORE:(b + 1) * S_PER_CORE],
                    rhs=at_sb[:], start=True, stop=True)
                VE.scalar_tensor_tensor(e_acc[:], e_acc[:], 1.0, eb_ps[:],
                                        OP.mult, OP.add)

        nc.sync.dma_start(eout, e_acc[:])

    nc.compile()
    return nc


_CACHE = {}


def _get_program(NT, NB):
    key = (NT, NB)
    if key not in _CACHE:
        _CACHE[key] = build_program(NT, NB)
    return _CACHE[key]


def run(inputs, trace=False, **kwargs):
    in_maps, NT, NB = _preprocess(inputs)
    nc = _get_program(NT, NB)
    res = run_bass_kernel_spmd(nc, in_maps, core_ids=list(range(N_CORES)),
                               trace=trace, **kwargs)
    out = np.concatenate([res.results[c]['eout'][:, 0] for c in range(N_CORES)])
    return out.astype(np.float32), res


def kernel(**inputs):
    out, _ = run(inputs)
    return out
